# revision 14
# baseline (speedup 1.0000x reference)
"""Distributed Trainium2 Bass kernel for nn_ActorGCN (GCN message passing).

Strategy (8 NeuronCores, nodes sharded across cores):
  The reference computes softmax(relu(BN(GCNConv(x)) @ W_lin)).  Because the
  GCN aggregation is linear, we aggregate FIRST on the 20-dim raw features
  (agg = A_norm @ x), then fold the whole 1024-wide hidden layer analytically:
  BatchNorm statistics of h = agg @ W + b are exact functions of the 21x21
  Gram matrix [agg,1]^T [agg,1], so the final output is
  softmax(relu(agg @ W_eff + b_eff)) with a tiny on-device-computed
  W_eff [20,2].  Per-core work: one dma_gather of source-node features
  (compacted per-core table, int16 indices), scale by the symmetric-norm
  edge coefficients, one-hot segmented-sum matmuls into PSUM (one 128-slot
  group per 128-row dst tile, guaranteed by degree-balanced node dealing),
  Gram accumulation, one small AllGather, stats fold, and the final
  [128,20]x[20,2] matmuls with fused relu/softmax.

Host-side prep is index-space only: degrees, norm coefficients, the
node->(core,tile,row) assignment, slot layouts and gather tables.
"""
import numpy as np
import ml_dtypes

F = 20
C = 8
P = 128
EPS = 1e-5
NQ = 4          # gather chunks
NSWQ = 4        # swdge queues (call k uses queue k % NSWQ)


# --------------------------------------------------------------------------
# host-side preprocessing (index space only)
# --------------------------------------------------------------------------
def _prep(state, edge_attr, edge_index, W_gcn, b_gcn, gamma, beta, W_lin, b_lin):
    N = state.shape[0] + edge_attr.shape[0]
    x_full = np.concatenate([np.asarray(state, np.float32),
                             np.asarray(edge_attr, np.float32)], axis=0)
    src = np.asarray(edge_index[0]).astype(np.int64)
    dst = np.asarray(edge_index[1]).astype(np.int64)

    deg_in = np.bincount(dst, minlength=N)
    deg = (deg_in + 1).astype(np.float32)
    dinv = (1.0 / np.sqrt(deg)).astype(np.float32)
    norm = (dinv[src] * dinv[dst]).astype(np.float32)
    dinv2 = (dinv * dinv).astype(np.float32)

    # degree-balanced node dealing over C*T bins of P rows each
    T = -(-N // (C * P))
    NB = C * T
    SHP = T * P
    order = np.argsort(-deg_in, kind="stable")
    nrounds = -(-N // NB)
    bin_of_node = np.empty(N, dtype=np.int64)
    for r in range(nrounds):
        lo, hi = r * NB, min((r + 1) * NB, N)
        seq = np.arange(hi - lo)
        b = seq if (r % 2 == 0) else (NB - 1 - seq)
        bin_of_node[order[lo:hi]] = b
    load = np.bincount(bin_of_node, weights=deg_in.astype(np.float64),
                       minlength=NB).astype(np.int64)
    if load.max() > P:
        zero_nodes = [list() for _ in range(NB)]
        for n in np.nonzero(deg_in == 0)[0]:
            zero_nodes[bin_of_node[n]].append(int(n))
        for b in np.nonzero(load > P)[0]:
            members = np.nonzero(bin_of_node == b)[0]
            members = list(members[np.argsort(deg_in[members])])
            while load[b] > P:
                pos = next(int(n) for n in members if deg_in[n] > 0)
                members.remove(pos)
                tgt = int(np.argmin(load + np.array(
                    [0 if zero_nodes[i] else 10**9 for i in range(NB)])))
                z = zero_nodes[tgt].pop()
                bin_of_node[pos], bin_of_node[z] = tgt, b
                zero_nodes[b].append(z)
                load[b] -= deg_in[pos]
                load[tgt] += deg_in[pos]
        assert load.max() <= P, load.max()

    ord2 = np.lexsort((np.arange(N), bin_of_node))
    row_in_bin = np.empty(N, dtype=np.int64)
    cnt_per_bin = np.bincount(bin_of_node, minlength=NB)
    assert cnt_per_bin.max() <= P
    starts = np.zeros(NB + 1, dtype=np.int64)
    np.cumsum(cnt_per_bin, out=starts[1:])
    row_in_bin[ord2] = np.arange(N) - starts[bin_of_node[ord2]]

    core_of_node = bin_of_node // T
    tile_of_node = bin_of_node % T
    slot_of_node = tile_of_node * P + row_in_bin

    node_at = np.full((C, SHP), -1, dtype=np.int64)
    node_at[core_of_node, slot_of_node] = np.arange(N)

    S = SHP
    ec = core_of_node[dst]
    et = tile_of_node[dst]
    erel = slot_of_node[dst] % P

    core_data = []
    U_list = []
    for c in range(C):
        m = ec == c
        s_c, t_c, rel_c, n_c = src[m], et[m], erel[m], norm[m]
        o = np.lexsort((s_c, t_c))
        s_c, t_c, rel_c, n_c = s_c[o], t_c[o], rel_c[o], n_c[o]
        uniq, idx_local = np.unique(s_c, return_inverse=True)
        U_list.append(len(uniq))
        core_data.append((s_c, t_c, rel_c, n_c, uniq, idx_local))
    U_pad = max(U_list)
    U_pad = -(-U_pad // 4) * 4
    assert U_pad < 32767, U_pad

    per_core = []
    for c in range(C):
        s_c, t_c, rel_c, n_c, uniq, idx_local = core_data[c]
        cnt = np.bincount(t_c, minlength=T)
        assert cnt.max() <= P
        cbase = np.zeros(T + 1, dtype=np.int64)
        np.cumsum(cnt, out=cbase[1:])
        slot = P * t_c + (np.arange(len(t_c)) - cbase[t_c])

        msg_idx = np.zeros(S, dtype=np.int16)
        msg_norm = np.zeros(S, dtype=np.float32)
        msg_dstrel = np.zeros(S, dtype=np.float32)
        msg_idx[slot] = idx_local.astype(np.int16)
        msg_norm[slot] = n_c
        msg_dstrel[slot] = rel_c.astype(np.float32)

        table = np.zeros((U_pad, 64), dtype=np.float32)
        table[:len(uniq), :F] = x_full[uniq]

        def slotted(a):
            return np.ascontiguousarray(a.reshape(S // P, P).T)

        # per-call idx wraps: call k covers slots [k*NI, (k+1)*NI)
        NI = 768
        KC = S // NI
        iws = []
        for k in range(KC):
            blk = msg_idx[k * NI:(k + 1) * NI]
            iws.append(np.tile(blk.reshape(NI // 16, 16).T, (8, 1)))
        iwrap = np.ascontiguousarray(np.stack(iws).reshape(KC * 128, NI // 16))

        nodes = node_at[c]
        ok = nodes >= 0
        xl = np.zeros((SHP, F), dtype=np.float32)
        xl[ok] = x_full[nodes[ok]]
        xl = np.ascontiguousarray(xl.reshape(T, P, F).transpose(1, 0, 2))
        d2 = np.zeros(SHP, dtype=np.float32)
        d2[ok] = dinv2[nodes[ok]]
        d2 = np.ascontiguousarray(d2.reshape(T, P).T)
        vv = np.ascontiguousarray(ok.astype(np.float32).reshape(T, P).T)

        per_core.append(dict(
            table=table, idx16=iwrap, normc=slotted(msg_norm),
            dstrel=slotted(msg_dstrel).astype(ml_dtypes.bfloat16),
            x_local=xl, dinv2=d2, valid=vv,
        ))

    H = W_gcn.shape[1]
    W1 = np.concatenate([np.asarray(W_gcn, np.float32),
                         np.asarray(b_gcn, np.float32)[None, :]], axis=0)
    WT8 = np.ascontiguousarray(np.asarray(W_gcn, np.float32).T
                               .reshape(8, P, F).transpose(1, 0, 2))
    def col8(v):
        return np.ascontiguousarray(np.asarray(v, np.float32).reshape(8, P).T)
    W_lin8 = np.ascontiguousarray(np.asarray(W_lin, np.float32)
                                  .reshape(8, P, 2).transpose(1, 0, 2))
    blin_pad = np.zeros((22, 2), dtype=np.float32)
    blin_pad[21, :] = np.asarray(b_lin, np.float32)

    SEL = np.zeros((22, 120), dtype=np.float32)
    for i in range(6):
        for a in range(20):
            SEL[a, 20 * i + a] = 1.0
    BMASK = np.zeros((120, 12), dtype=np.float32)
    for i in range(6):
        BMASK[20 * i:20 * (i + 1), 2 * i:2 * (i + 1)] = 1.0
    SEL3 = np.zeros((22, 128), dtype=np.float32)
    SEL3[20, :] = 1.0
    SEL3[21, :] = 1.0
    iota_bc = np.tile(np.arange(P, dtype=np.float32)[None, :], (P, 1)) \
        .astype(ml_dtypes.bfloat16)
    identity = np.eye(P, dtype=np.float32)

    shared = dict(W1=W1, WT8=WT8, bcol8=col8(b_gcn), beta8=col8(beta),
                  gamma8=col8(gamma), W_lin8=W_lin8, blin_pad=blin_pad,
                  SEL=SEL, SEL3=SEL3, BMASK=BMASK, iota_bc=iota_bc,
                  identity=identity)
    meta = dict(N=N, T=T, SHP=SHP, S=S, U_pad=U_pad, H=H,
                core_of_node=core_of_node, slot_of_node=slot_of_node)
    return per_core, shared, meta


# --------------------------------------------------------------------------
# device kernel
# --------------------------------------------------------------------------
def _build(meta, debug=False):
    import concourse.bass as bass
    import concourse.bacc as bacc
    import concourse.mybir as mybir
    from concourse.tile import TileContext

    f32 = mybir.dt.float32
    bf16 = mybir.dt.bfloat16
    i16 = mybir.dt.int16
    T, S, U_pad, N = meta["T"], meta["S"], meta["U_pad"], meta["N"]
    G6 = T // 6                      # 6-tile groups (34)
    AX = mybir.AxisListType.X
    OP = mybir.AluOpType
    ACT = mybir.ActivationFunctionType

    nc = bacc.Bacc(None, target_bir_lowering=False,
                   num_swdge_queues=NSWQ)

    def inp(name, shape, dt=f32):
        return nc.declare_dram_parameter(name, list(shape), dt, isOutput=False)

    table = inp("table", [U_pad, 64])
    NI = 768                     # idxs per dma_gather call
    KC = S // NI                 # calls (34), one per 6-tile group
    idx16 = inp("idx16", [KC * P, NI // 16], i16)
    normc = inp("normc", [P, T])
    dstrel = inp("dstrel", [P, T], bf16)
    x_local = inp("x_local", [P, T * F])
    dinv2 = inp("dinv2", [P, T])
    valid = inp("valid", [P, T])
    W1 = inp("W1", [21, 1024])
    WT8 = inp("WT8", [P, 8 * F])
    bcol8 = inp("bcol8", [P, 8])
    beta8 = inp("beta8", [P, 8])
    gamma8 = inp("gamma8", [P, 8])
    W_lin8 = inp("W_lin8", [P, 16])
    blin_pad = inp("blin_pad", [22, 2])
    SEL = inp("SEL", [22, 120])
    SEL3 = inp("SEL3", [22, 128])
    BMASK = inp("BMASK", [120, 12])
    iota_bc = inp("iota_bc", [P, P], bf16)
    identity = inp("identity", [P, P])
    out_ext = nc.declare_dram_parameter("out", [S, 2], f32, isOutput=True)
    if debug:
        dbg_aggp = nc.declare_dram_parameter("dbg_aggp", [P, T * F], f32,
                                             isOutput=True)
        dbg_g1 = nc.declare_dram_parameter("dbg_g1", [21, 21], f32,
                                           isOutput=True)
        dbg_wstk = nc.declare_dram_parameter("dbg_wstk", [120, 12], f32,
                                             isOutput=True)
        dbg_beff = nc.declare_dram_parameter("dbg_beff", [P, 12], f32,
                                             isOutput=True)
        dbg_logit = nc.declare_dram_parameter("dbg_logit", [P, T * 2], f32,
                                              isOutput=True)
        dbg_msgs = nc.declare_dram_parameter("dbg_msgs", [P, T * F], f32,
                                             isOutput=True)

    with TileContext(nc) as tc:
        with (
            tc.tile_pool(name="dram", bufs=1, space="DRAM") as dpool,
            tc.tile_pool(name="const", bufs=1) as cpool,
            tc.tile_pool(name="big", bufs=1) as bpool,
            tc.tile_pool(name="graw", bufs=2) as gpool,
            tc.tile_pool(name="oh", bufs=2) as ohpool,
            tc.tile_pool(name="small", bufs=2) as spool,
        ):
            ag_in = dpool.tile([6, 21, 21], f32, tag="ag_in", name="ag_in")
            ag_out = dpool.tile([8, 6, 21, 21], f32, tag="ag_out",
                                name="ag_out", addr_space="Shared")

            # ---- load inputs ----
            def load(nm, ap, shape, dt=f32, pool=cpool):
                t = pool.tile(list(shape), dt, tag=nm, name=nm)
                nc.sync.dma_start(out=t[:], in_=ap[:])
                return t

            idxall_t = cpool.tile([P, KC * (NI // 16)], i16, tag="idxall",
                                  name="idxall")
            nc.sync.dma_start(
                out=idxall_t[:].rearrange("p (k w) -> p k w", w=NI // 16),
                in_=idx16[:].rearrange("(k p) w -> p k w", p=P))
            norm_t = load("norm_t", normc, [P, T])
            dstrel_t = load("dstrel_t", dstrel, [P, T], bf16)
            xl_t = load("xl_t", x_local, [P, T * F], pool=bpool)
            d2_t = load("d2_t", dinv2, [P, T])
            valid_t = load("valid_t", valid, [P, T])
            W1_t = load("W1_t", W1, [21, 1024])
            WT8_t = load("WT8_t", WT8, [P, 8 * F])
            bcol8_t = load("bcol8_t", bcol8, [P, 8])
            beta8_t = load("beta8_t", beta8, [P, 8])
            gamma8_t = load("gamma8_t", gamma8, [P, 8])
            Wlin8_t = load("Wlin8_t", W_lin8, [P, 16])
            blin_t = load("blin_t", blin_pad, [22, 2])
            SEL_t = load("SEL_t", SEL, [22, 120])
            SEL3_t = load("SEL3_t", SEL3, [22, 128])
            bmask_t = load("bmask_t", BMASK, [120, 12])
            iota_t = load("iota_t", iota_bc, [P, P], bf16)
            ident_t = load("ident_t", identity, [P, P])

            # ---- self-loop term (in place into xl_t) ----
            nc.vector.tensor_tensor(
                out=xl_t[:].rearrange("p (t f) -> p t f", f=F),
                in0=xl_t[:].rearrange("p (t f) -> p t f", f=F),
                in1=d2_t[:][:, :, None].to_broadcast([P, T, F]),
                op=OP.mult)

            # ---- agg via gather + one-hot segmented-sum matmuls ----
            # One dma_gather call per 6-tile group (NI=768 idxs; the SWDGE
            # descriptor ring caps ~1k descriptors per instruction).
            # agg_t: [tile|valid]-interleaved (21 cols/tile) for the Gram;
            # aggp_t: packed 20 cols/tile for the PE transposes.
            agg_t = bpool.tile([P, T * 21], f32)
            aggp_t = bpool.tile([P, T * F], f32)
            nc.vector.tensor_copy(
                out=agg_t[:].rearrange("p (t u) -> p t u", u=21)[:, :, 20:21],
                in_=valid_t[:][:, :, None])
            p6ctx = tc.tile_pool(name="p6", bufs=3, space="PSUM")
            p6pool = p6ctx.__enter__()
            for k in range(KC):
                itk = spool.tile([P, NI // 16], i16, tag="itk", bufs=4,
                                 name=f"itk_{k}")
                nc.vector.tensor_copy(
                    out=itk[:],
                    in_=idxall_t[:, k * (NI // 16):(k + 1) * (NI // 16)])
                graw = gpool.tile([P, 6 * 64], f32, tag="graw", bufs=4,
                                  name=f"graw_{k}")
                nc.gpsimd.dma_gather(
                    out_ap=graw[:].rearrange("p (n e) -> p n e", e=64),
                    in_ap=table[:],
                    idxs_ap=itk[:],
                    num_idxs=NI,
                    num_idxs_reg=NI,
                    elem_size=64,
                    queue_num=k % NSWQ,
                )
                msgs = gpool.tile([P, 6 * F], bf16, tag="msgs", bufs=4,
                                  name=f"msgs_{k}")
                nc.vector.tensor_tensor(
                    out=msgs[:].rearrange("p (t f) -> p t f", f=F),
                    in0=graw[:].rearrange("p (n e) -> p n e", e=64)[:, :, 0:F],
                    in1=norm_t[:, k * 6:(k + 1) * 6][:, :, None]
                        .to_broadcast([P, 6, F]),
                    op=OP.mult)
                oh = ohpool.tile([P, 6 * P], bf16, tag="oh", bufs=4,
                                 name=f"oh_{k}")
                nc.vector.tensor_tensor(
                    out=oh[:].rearrange("p (t q) -> p t q", q=P),
                    in0=dstrel_t[:, k * 6:(k + 1) * 6][:, :, None]
                        .to_broadcast([P, 6, P]),
                    in1=iota_t[:][:, None, :].to_broadcast([P, 6, P]),
                    op=OP.is_equal)
                ps6 = p6pool.tile([P, 120], f32, tag="ps6", name=f"ps6_{k}")
                for sl in range(6):
                    nc.tensor.matmul(
                        out=ps6[:, sl * F:(sl + 1) * F],
                        lhsT=oh[:, sl * P:(sl + 1) * P],
                        rhs=msgs[:, sl * F:(sl + 1) * F],
                        start=True, stop=True)
                nc.vector.tensor_tensor(
                    out=agg_t[:, k * 126:(k + 1) * 126]
                        .rearrange("p (s u) -> p s u", u=21)[:, :, 0:F],
                    in0=ps6[:].rearrange("p (s f) -> p s f", f=F),
                    in1=xl_t[:, k * 120:(k + 1) * 120]
                        .rearrange("p (s f) -> p s f", f=F),
                    op=OP.add)
                nc.vector.tensor_tensor(
                    out=aggp_t[:, k * 120:(k + 1) * 120],
                    in0=ps6[:],
                    in1=xl_t[:, k * 120:(k + 1) * 120],
                    op=OP.add)

            p6ctx.__exit__(None, None, None)

            # ---- Gram accumulation ----
            ggctx = tc.tile_pool(name="pgg", bufs=1, space="PSUM")
            ggpool = ggctx.__enter__()
            trctx = tc.tile_pool(name="ptr", bufs=2, space="PSUM")
            trpool = trctx.__enter__()
            gg_ps = ggpool.tile([126, 126], f32)
            for m in range(G6):
                nc.tensor.matmul(
                    out=gg_ps[:],
                    lhsT=agg_t[:, m * 126:(m + 1) * 126],
                    rhs=agg_t[:, m * 126:(m + 1) * 126],
                    start=(m == 0), stop=(m == G6 - 1))
            gg_sb = spool.tile([126, 126], f32)
            nc.vector.tensor_copy(out=gg_sb[:], in_=gg_ps[:])
            for i in range(6):
                nc.sync.dma_start(
                    out=ag_in[i],
                    in_=gg_sb[21 * i:21 * (i + 1), 21 * i:21 * i + 21])


            # ---- AllGather of Gram partials (issue early; overlaps PE) ----
            nc.gpsimd.collective_compute(
                "AllGather", OP.bypass,
                replica_groups=[list(range(C))],
                ins=[ag_in[:].opt()], outs=[ag_out[:].opt()])

            # ---- transposes for the final matmuls (independent of stats) ----
            trm_all = bpool.tile([120, G6 * P], f32)
            for m in range(G6):
                tr_ps = trpool.tile([120, P], f32, tag="trps", name=f"trps_{m}")
                nc.tensor.transpose(
                    out=tr_ps[:],
                    in_=aggp_t[:, m * 120:(m + 1) * 120],
                    identity=ident_t[:])
                nc.vector.tensor_copy(
                    out=trm_all[:, m * P:(m + 1) * P], in_=tr_ps[:])

            trctx.__exit__(None, None, None)
            ggctx.__exit__(None, None, None)

            # ---- fold AllGather result ----
            stctx = tc.tile_pool(name="pst", bufs=1, space="PSUM")
            stpool = stctx.__enter__()
            mpctx = tc.tile_pool(name="pmp", bufs=2, space="PSUM")
            mppool = mpctx.__enter__()
            lgctx = tc.tile_pool(name="plg", bufs=2, space="PSUM")
            lgpool = lgctx.__enter__()
            gsum_t = spool.tile([21, 48 * 21], f32)
            nc.sync.dma_start(
                out=gsum_t[:].rearrange("a (k b) -> a k b", b=21),
                in_=ag_out[:].rearrange("c s a b -> a (c s) b"))
            G1_t = spool.tile([21, 21], f32)
            nc.vector.reduce_sum(
                out=G1_t[:],
                in_=gsum_t[:].rearrange("a (k b) -> a b k", b=21),
                axis=AX)

            # ---- BN stats -> W_eff/b_eff ----
            w1aug_t = spool.tile([P, 8 * 21], f32)
            nc.vector.tensor_copy(
                out=w1aug_t[:].rearrange("p (c u) -> p c u", u=21)[:, :, 0:F],
                in_=WT8_t[:].rearrange("p (c f) -> p c f", f=F))
            nc.vector.tensor_copy(
                out=w1aug_t[:].rearrange("p (c u) -> p c u", u=21)[:, :, 20:21],
                in_=bcol8_t[:][:, :, None])
            wb_ps = stpool.tile([22, 2], f32, tag="wb", bufs=1)
            for c8 in range(8):
                m_ps = mppool.tile([P, 21], f32, tag="mps", name=f"mps_{c8}")
                nc.tensor.matmul(
                    out=m_ps[:], lhsT=W1_t[:, c8 * P:(c8 + 1) * P],
                    rhs=G1_t[:], start=True, stop=True)
                prod = spool.tile([P, 21], f32, tag="prod", name=f"prod_{c8}")
                nc.vector.tensor_tensor(
                    out=prod[:], in0=m_ps[:],
                    in1=w1aug_t[:, c8 * 21:(c8 + 1) * 21], op=OP.mult)
                ex2 = spool.tile([P, 1], f32, tag="ex2", name=f"ex2_{c8}")
                nc.vector.reduce_sum(out=ex2[:], in_=prod[:], axis=AX)
                mean = spool.tile([P, 1], f32, tag="mean", name=f"mean_{c8}")
                nc.vector.tensor_scalar_mul(
                    out=mean[:], in0=m_ps[:, 20:21], scalar1=1.0 / N)
                mm2 = spool.tile([P, 1], f32, tag="mm2", name=f"mm2_{c8}")
                nc.vector.tensor_tensor(
                    out=mm2[:], in0=mean[:], in1=mean[:], op=OP.mult)
                ex2n = spool.tile([P, 1], f32, tag="ex2n", name=f"ex2n_{c8}")
                nc.vector.tensor_scalar_mul(
                    out=ex2n[:], in0=ex2[:], scalar1=1.0 / N)
                var = spool.tile([P, 1], f32, tag="var", name=f"var_{c8}")
                nc.vector.tensor_tensor(
                    out=var[:], in0=ex2n[:], in1=mm2[:], op=OP.subtract)
                nc.vector.tensor_scalar_add(out=var[:], in0=var[:],
                                            scalar1=EPS)
                sd = spool.tile([P, 1], f32, tag="sd", name=f"sd_{c8}")
                nc.scalar.activation(out=sd[:], in_=var[:], func=ACT.Sqrt)
                dsc = spool.tile([P, 1], f32, tag="dsc", name=f"dsc_{c8}")
                nc.vector.reciprocal(out=dsc[:], in_=sd[:])
                nc.vector.tensor_tensor(
                    out=dsc[:], in0=dsc[:], in1=gamma8_t[:, c8:c8 + 1],
                    op=OP.mult)
                aug = spool.tile([P, 22], f32, tag="aug", name=f"aug_{c8}")
                nc.vector.tensor_scalar_mul(
                    out=aug[:, 0:F],
                    in0=WT8_t[:, c8 * F:(c8 + 1) * F], scalar1=dsc[:])
                bm = spool.tile([P, 1], f32, tag="bm", name=f"bm_{c8}")
                nc.vector.tensor_tensor(
                    out=bm[:], in0=bcol8_t[:, c8:c8 + 1], in1=mean[:],
                    op=OP.subtract)
                nc.vector.tensor_tensor(
                    out=aug[:, 20:21], in0=bm[:], in1=dsc[:], op=OP.mult)
                nc.vector.tensor_copy(
                    out=aug[:, 21:22], in_=beta8_t[:, c8:c8 + 1])
                nc.tensor.matmul(
                    out=wb_ps[:], lhsT=aug[:],
                    rhs=Wlin8_t[:, 2 * c8:2 * c8 + 2],
                    start=(c8 == 0), stop=(c8 == 7))
            rhs2 = spool.tile([22, 2], f32)
            nc.vector.tensor_tensor(
                out=rhs2[:], in0=wb_ps[:], in1=blin_t[:], op=OP.add)
            rhs_tiled = spool.tile([22, 12], f32)
            nc.vector.tensor_copy(
                out=rhs_tiled[:].rearrange("p (i o) -> p i o", o=2),
                in_=rhs2[:][:, None, :].to_broadcast([22, 6, 2]))
            wstack_ps = stpool.tile([120, 12], f32, tag="wstk", bufs=1)
            nc.tensor.matmul(out=wstack_ps[:], lhsT=SEL_t[:], rhs=rhs_tiled[:],
                             start=True, stop=True)
            wstack_t = spool.tile([120, 12], f32)
            nc.vector.tensor_tensor(out=wstack_t[:], in0=wstack_ps[:],
                                    in1=bmask_t[:], op=OP.mult)
            beff_ps = stpool.tile([P, 12], f32, tag="beff", bufs=1)
            nc.tensor.matmul(out=beff_ps[:], lhsT=SEL3_t[:], rhs=rhs_tiled[:],
                             start=True, stop=True)
            beff_t = spool.tile([P, 12], f32)
            nc.vector.tensor_copy(out=beff_t[:], in_=beff_ps[:])

            # ---- final matmuls + fused relu/softmax ----
            logits_t = bpool.tile([P, G6 * 12], f32)
            for m in range(G6):
                log_ps = lgpool.tile([P, 12], f32, tag="logps",
                                     name=f"logps_{m}")
                nc.tensor.matmul(out=log_ps[:],
                                 lhsT=trm_all[:, m * P:(m + 1) * P],
                                 rhs=wstack_t[:], start=True, stop=True)
                nc.vector.tensor_tensor(
                    out=logits_t[:, m * 12:(m + 1) * 12],
                    in0=log_ps[:], in1=beff_t[:], op=OP.add)
            esb = bpool.tile([P, G6 * 12], f32)
            nc.scalar.activation(out=esb[:], in_=logits_t[:], func=ACT.Exp)
            nc.vector.tensor_scalar_max(out=esb[:], in0=esb[:], scalar1=1.0)
            psum_t = spool.tile([P, T], f32)
            nc.vector.tensor_tensor(
                out=psum_t[:],
                in0=esb[:].rearrange("p (t o) -> p t o", o=2)[:, :, 0:1],
                in1=esb[:].rearrange("p (t o) -> p t o", o=2)[:, :, 1:2],
                op=OP.add)
            rc = spool.tile([P, T], f32)
            nc.vector.reciprocal(out=rc[:], in_=psum_t[:])
            outv = bpool.tile([P, T * 2], f32)
            nc.vector.tensor_tensor(
                out=outv[:].rearrange("p (t o) -> p t o", o=2),
                in0=esb[:].rearrange("p (t o) -> p t o", o=2),
                in1=rc[:][:, :, None].to_broadcast([P, T, 2]),
                op=OP.mult)
            nc.sync.dma_start(
                out=out_ext[:].rearrange("(t p) o -> p t o", p=P),
                in_=outv[:].rearrange("p (t o) -> p t o", o=2))
            if debug:
                nc.sync.dma_start(out=dbg_aggp[:], in_=aggp_t[:])
                nc.sync.dma_start(out=dbg_g1[:], in_=G1_t[:])
                nc.sync.dma_start(out=dbg_wstk[:], in_=wstack_t[:])
                nc.sync.dma_start(out=dbg_beff[:], in_=beff_t[:])
                nc.sync.dma_start(out=dbg_logit[:], in_=logits_t[:])
                nc.sync.dma_start(out=dbg_msgs[:], in_=aggp_t[:])
            lgctx.__exit__(None, None, None)
            mpctx.__exit__(None, None, None)
            stctx.__exit__(None, None, None)

    nc.finalize()
    return nc


# --------------------------------------------------------------------------
# entry point
# --------------------------------------------------------------------------
TRACE = False           # set True (e.g. from test.py) to neuron-profile the run
LAST_EXEC_NS = None


def kernel(**inputs):
    global LAST_EXEC_NS
    from concourse.bass_utils import run_bass_kernel_spmd

    per_core, shared, meta = _prep(**inputs)
    nc = _build(meta)
    in_maps = []
    for c in range(C):
        d = dict(per_core[c])
        m = {
            "table": d["table"], "idx16": d["idx16"], "normc": d["normc"],
            "dstrel": d["dstrel"],
            "x_local": np.ascontiguousarray(
                d["x_local"].reshape(P, meta["T"] * F)),
            "dinv2": d["dinv2"], "valid": d["valid"],
            "W1": shared["W1"],
            "WT8": np.ascontiguousarray(shared["WT8"].reshape(P, 8 * F)),
            "bcol8": shared["bcol8"], "beta8": shared["beta8"],
            "gamma8": shared["gamma8"],
            "W_lin8": np.ascontiguousarray(shared["W_lin8"].reshape(P, 16)),
            "blin_pad": shared["blin_pad"], "SEL": shared["SEL"],
            "SEL3": shared["SEL3"], "BMASK": shared["BMASK"],
            "iota_bc": shared["iota_bc"],
            "identity": shared["identity"],
        }
        in_maps.append(m)
    res = run_bass_kernel_spmd(nc, in_maps, core_ids=list(range(C)),
                               trace=TRACE)
    LAST_EXEC_NS = res.exec_time_ns
    outs = [res.results[c]["out"] for c in range(C)]
    stacked = np.stack(outs)
    full = stacked[meta["core_of_node"], meta["slot_of_node"]]
    return np.ascontiguousarray(full.astype(np.float32))


# revision 16
# speedup vs baseline: 1.5037x; 1.5037x over previous
"""Distributed Trainium2 Bass kernel for nn_ActorGCN (GCN message passing).

Strategy (8 NeuronCores, nodes sharded across cores):
  The reference computes softmax(relu(BN(GCNConv(x)) @ W_lin)).  Because the
  GCN aggregation is linear, we aggregate FIRST on the 20-dim raw features
  (agg = A_norm @ x), then fold the whole 1024-wide hidden layer analytically:
  BatchNorm statistics of h = agg @ W + b are exact functions of the 21x21
  Gram matrix [agg,1]^T [agg,1], so the final output is
  softmax(relu(agg @ W_eff + b_eff)) with a tiny on-device-computed
  W_eff [20,2].  Per-core work: one dma_gather of source-node features
  (compacted per-core table, int16 indices), scale by the symmetric-norm
  edge coefficients, one-hot segmented-sum matmuls into PSUM (one 128-slot
  group per 128-row dst tile, guaranteed by degree-balanced node dealing),
  Gram accumulation, one small AllGather, stats fold, and the final
  [128,20]x[20,2] matmuls with fused relu/softmax.

Host-side prep is index-space only: degrees, norm coefficients, the
node->(core,tile,row) assignment, slot layouts and gather tables.
"""
import numpy as np
import ml_dtypes

F = 20
C = 8
P = 128
EPS = 1e-5
NQ = 4          # gather chunks
NSWQ = 4        # swdge queues (call k uses queue k % NSWQ)


# --------------------------------------------------------------------------
# host-side preprocessing (index space only)
# --------------------------------------------------------------------------
def _prep(state, edge_attr, edge_index, W_gcn, b_gcn, gamma, beta, W_lin, b_lin):
    N = state.shape[0] + edge_attr.shape[0]
    x_full = np.concatenate([np.asarray(state, np.float32),
                             np.asarray(edge_attr, np.float32)], axis=0)
    src = np.asarray(edge_index[0]).astype(np.int64)
    dst = np.asarray(edge_index[1]).astype(np.int64)

    deg_in = np.bincount(dst, minlength=N)
    deg = (deg_in + 1).astype(np.float32)
    dinv = (1.0 / np.sqrt(deg)).astype(np.float32)
    norm = (dinv[src] * dinv[dst]).astype(np.float32)
    dinv2 = (dinv * dinv).astype(np.float32)

    # degree-balanced node dealing over C*T bins of P rows each
    T = -(-N // (C * P))
    NB = C * T
    SHP = T * P
    order = np.argsort(-deg_in, kind="stable")
    nrounds = -(-N // NB)
    bin_of_node = np.empty(N, dtype=np.int64)
    for r in range(nrounds):
        lo, hi = r * NB, min((r + 1) * NB, N)
        seq = np.arange(hi - lo)
        b = seq if (r % 2 == 0) else (NB - 1 - seq)
        bin_of_node[order[lo:hi]] = b
    load = np.bincount(bin_of_node, weights=deg_in.astype(np.float64),
                       minlength=NB).astype(np.int64)
    if load.max() > P:
        zero_nodes = [list() for _ in range(NB)]
        for n in np.nonzero(deg_in == 0)[0]:
            zero_nodes[bin_of_node[n]].append(int(n))
        for b in np.nonzero(load > P)[0]:
            members = np.nonzero(bin_of_node == b)[0]
            members = list(members[np.argsort(deg_in[members])])
            while load[b] > P:
                pos = next(int(n) for n in members if deg_in[n] > 0)
                members.remove(pos)
                tgt = int(np.argmin(load + np.array(
                    [0 if zero_nodes[i] else 10**9 for i in range(NB)])))
                z = zero_nodes[tgt].pop()
                bin_of_node[pos], bin_of_node[z] = tgt, b
                zero_nodes[b].append(z)
                load[b] -= deg_in[pos]
                load[tgt] += deg_in[pos]
        assert load.max() <= P, load.max()

    ord2 = np.lexsort((np.arange(N), bin_of_node))
    row_in_bin = np.empty(N, dtype=np.int64)
    cnt_per_bin = np.bincount(bin_of_node, minlength=NB)
    assert cnt_per_bin.max() <= P
    starts = np.zeros(NB + 1, dtype=np.int64)
    np.cumsum(cnt_per_bin, out=starts[1:])
    row_in_bin[ord2] = np.arange(N) - starts[bin_of_node[ord2]]

    core_of_node = bin_of_node // T
    tile_of_node = bin_of_node % T
    slot_of_node = tile_of_node * P + row_in_bin

    node_at = np.full((C, SHP), -1, dtype=np.int64)
    node_at[core_of_node, slot_of_node] = np.arange(N)

    S = SHP
    ec = core_of_node[dst]
    et = tile_of_node[dst]
    erel = slot_of_node[dst] % P

    core_data = []
    U_list = []
    for c in range(C):
        m = ec == c
        s_c, t_c, rel_c, n_c = src[m], et[m], erel[m], norm[m]
        o = np.lexsort((s_c, t_c))
        s_c, t_c, rel_c, n_c = s_c[o], t_c[o], rel_c[o], n_c[o]
        uniq, idx_local = np.unique(s_c, return_inverse=True)
        U_list.append(len(uniq))
        core_data.append((s_c, t_c, rel_c, n_c, uniq, idx_local))
    U_pad = max(U_list)
    U_pad = -(-U_pad // 4) * 4
    assert U_pad < 32767, U_pad

    per_core = []
    for c in range(C):
        s_c, t_c, rel_c, n_c, uniq, idx_local = core_data[c]
        cnt = np.bincount(t_c, minlength=T)
        assert cnt.max() <= P
        cbase = np.zeros(T + 1, dtype=np.int64)
        np.cumsum(cnt, out=cbase[1:])
        slot = P * t_c + (np.arange(len(t_c)) - cbase[t_c])

        msg_idx = np.zeros(S, dtype=np.int16)
        msg_norm = np.zeros(S, dtype=np.float32)
        msg_dstrel = np.zeros(S, dtype=np.float32)
        msg_idx[slot] = idx_local.astype(np.int16)
        msg_norm[slot] = n_c
        msg_dstrel[slot] = rel_c.astype(np.float32)

        table = np.zeros((U_pad, 64), dtype=np.float32)
        table[:len(uniq), :F] = x_full[uniq]

        def slotted(a):
            return np.ascontiguousarray(a.reshape(S // P, P).T)

        # per-call idx wraps: call k covers slots [k*NI, (k+1)*NI)
        NI = 768
        KC = S // NI
        iws = []
        for k in range(KC):
            blk = msg_idx[k * NI:(k + 1) * NI]
            iws.append(np.tile(blk.reshape(NI // 16, 16).T, (8, 1)))
        iwrap = np.ascontiguousarray(np.stack(iws).reshape(KC * 128, NI // 16))

        nodes = node_at[c]
        ok = nodes >= 0
        xl = np.zeros((SHP, F), dtype=np.float32)
        xl[ok] = x_full[nodes[ok]]
        xl = np.ascontiguousarray(xl.reshape(T, P, F).transpose(1, 0, 2))
        d2 = np.zeros(SHP, dtype=np.float32)
        d2[ok] = dinv2[nodes[ok]]
        d2 = np.ascontiguousarray(d2.reshape(T, P).T)
        vv = np.ascontiguousarray(ok.astype(np.float32).reshape(T, P).T)

        per_core.append(dict(
            table=table, idx16=iwrap, normc=slotted(msg_norm),
            dstrel=slotted(msg_dstrel).astype(ml_dtypes.bfloat16),
            x_local=xl, dinv2=d2, valid=vv,
        ))

    H = W_gcn.shape[1]
    W1 = np.concatenate([np.asarray(W_gcn, np.float32),
                         np.asarray(b_gcn, np.float32)[None, :]], axis=0)
    WT8 = np.ascontiguousarray(np.asarray(W_gcn, np.float32).T
                               .reshape(8, P, F).transpose(1, 0, 2))
    def col8(v):
        return np.ascontiguousarray(np.asarray(v, np.float32).reshape(8, P).T)
    W_lin8 = np.ascontiguousarray(np.asarray(W_lin, np.float32)
                                  .reshape(8, P, 2).transpose(1, 0, 2))
    blin_pad = np.zeros((22, 2), dtype=np.float32)
    blin_pad[21, :] = np.asarray(b_lin, np.float32)

    SEL = np.zeros((22, 120), dtype=np.float32)
    for i in range(6):
        for a in range(20):
            SEL[a, 20 * i + a] = 1.0
    BMASK = np.zeros((120, 12), dtype=np.float32)
    for i in range(6):
        BMASK[20 * i:20 * (i + 1), 2 * i:2 * (i + 1)] = 1.0
    SEL3 = np.zeros((22, 128), dtype=np.float32)
    SEL3[20, :] = 1.0
    SEL3[21, :] = 1.0
    iota_bc = np.tile(np.arange(P, dtype=np.float32)[None, :], (P, 1)) \
        .astype(ml_dtypes.bfloat16)
    identity = np.eye(P, dtype=np.float32)

    shared = dict(W1=W1, WT8=WT8, bcol8=col8(b_gcn), beta8=col8(beta),
                  gamma8=col8(gamma), W_lin8=W_lin8, blin_pad=blin_pad,
                  SEL=SEL, SEL3=SEL3, BMASK=BMASK, iota_bc=iota_bc,
                  identity=identity)
    meta = dict(N=N, T=T, SHP=SHP, S=S, U_pad=U_pad, H=H,
                core_of_node=core_of_node, slot_of_node=slot_of_node)
    return per_core, shared, meta


# --------------------------------------------------------------------------
# device kernel
# --------------------------------------------------------------------------
def _build(meta, debug=False):
    import concourse.bass as bass
    import concourse.bacc as bacc
    import concourse.mybir as mybir
    from concourse.tile import TileContext

    f32 = mybir.dt.float32
    bf16 = mybir.dt.bfloat16
    i16 = mybir.dt.int16
    T, S, U_pad, N = meta["T"], meta["S"], meta["U_pad"], meta["N"]
    G6 = T // 6                      # 6-tile groups (34)
    AX = mybir.AxisListType.X
    OP = mybir.AluOpType
    ACT = mybir.ActivationFunctionType

    nc = bacc.Bacc(None, target_bir_lowering=False,
                   num_swdge_queues=NSWQ)

    def inp(name, shape, dt=f32):
        return nc.declare_dram_parameter(name, list(shape), dt, isOutput=False)

    table = inp("table", [U_pad, 64])
    NI = 768                     # idxs per dma_gather call
    KC = S // NI                 # calls (34), one per 6-tile group
    idx16 = inp("idx16", [KC * P, NI // 16], i16)
    normc = inp("normc", [P, T])
    dstrel = inp("dstrel", [P, T], bf16)
    x_local = inp("x_local", [P, T * F])
    dinv2 = inp("dinv2", [P, T])
    valid = inp("valid", [P, T])
    W1 = inp("W1", [21, 1024])
    WT8 = inp("WT8", [P, 8 * F])
    bcol8 = inp("bcol8", [P, 8])
    beta8 = inp("beta8", [P, 8])
    gamma8 = inp("gamma8", [P, 8])
    W_lin8 = inp("W_lin8", [P, 16])
    blin_pad = inp("blin_pad", [22, 2])
    SEL = inp("SEL", [22, 120])
    SEL3 = inp("SEL3", [22, 128])
    BMASK = inp("BMASK", [120, 12])
    iota_bc = inp("iota_bc", [P, P], bf16)
    identity = inp("identity", [P, P])
    out_ext = nc.declare_dram_parameter("out", [S, 2], f32, isOutput=True)
    if debug:
        dbg_aggp = nc.declare_dram_parameter("dbg_aggp", [P, T * F], f32,
                                             isOutput=True)
        dbg_g1 = nc.declare_dram_parameter("dbg_g1", [21, 21], f32,
                                           isOutput=True)
        dbg_wstk = nc.declare_dram_parameter("dbg_wstk", [120, 12], f32,
                                             isOutput=True)
        dbg_beff = nc.declare_dram_parameter("dbg_beff", [P, 12], f32,
                                             isOutput=True)
        dbg_logit = nc.declare_dram_parameter("dbg_logit", [P, T * 2], f32,
                                              isOutput=True)
        dbg_msgs = nc.declare_dram_parameter("dbg_msgs", [P, T * F], f32,
                                             isOutput=True)

    with TileContext(nc) as tc:
        with (
            tc.tile_pool(name="dram", bufs=1, space="DRAM") as dpool,
            tc.tile_pool(name="const", bufs=1) as cpool,
            tc.tile_pool(name="big", bufs=1) as bpool,
            tc.tile_pool(name="graw", bufs=2) as gpool,
            tc.tile_pool(name="oh", bufs=2) as ohpool,
            tc.tile_pool(name="small", bufs=2) as spool,
        ):
            ag_in = dpool.tile([6, 21, 21], f32, tag="ag_in", name="ag_in")
            ag_out = dpool.tile([8, 6, 21, 21], f32, tag="ag_out",
                                name="ag_out", addr_space="Shared")

            # ---- load inputs ----
            def load(nm, ap, shape, dt=f32, pool=cpool):
                t = pool.tile(list(shape), dt, tag=nm, name=nm)
                nc.sync.dma_start(out=t[:], in_=ap[:])
                return t


            norm_t = load("norm_t", normc, [P, T])
            dstrel_t = load("dstrel_t", dstrel, [P, T], bf16)
            xl_t = load("xl_t", x_local, [P, T * F], pool=bpool)
            d2_t = load("d2_t", dinv2, [P, T])
            valid_t = load("valid_t", valid, [P, T])
            W1_t = load("W1_t", W1, [21, 1024])
            WT8_t = load("WT8_t", WT8, [P, 8 * F])
            bcol8_t = load("bcol8_t", bcol8, [P, 8])
            beta8_t = load("beta8_t", beta8, [P, 8])
            gamma8_t = load("gamma8_t", gamma8, [P, 8])
            Wlin8_t = load("Wlin8_t", W_lin8, [P, 16])
            blin_t = load("blin_t", blin_pad, [22, 2])
            SEL_t = load("SEL_t", SEL, [22, 120])
            SEL3_t = load("SEL3_t", SEL3, [22, 128])
            bmask_t = load("bmask_t", BMASK, [120, 12])
            iota_t = load("iota_t", iota_bc, [P, P], bf16)
            ident_t = load("ident_t", identity, [P, P])

            # ---- self-loop term (in place into xl_t) ----
            nc.vector.tensor_tensor(
                out=xl_t[:].rearrange("p (t f) -> p t f", f=F),
                in0=xl_t[:].rearrange("p (t f) -> p t f", f=F),
                in1=d2_t[:][:, :, None].to_broadcast([P, T, F]),
                op=OP.mult)

            # ---- agg via gather + one-hot segmented-sum matmuls ----
            # One dma_gather call per 6-tile group (NI=768 idxs; the SWDGE
            # descriptor ring caps ~1k descriptors per instruction).
            # agg_t: [tile|valid]-interleaved (21 cols/tile) for the Gram;
            # aggp_t: packed 20 cols/tile for the PE transposes.
            agg_t = bpool.tile([P, T * 21], f32)
            aggp_t = bpool.tile([P, T * F], f32)
            nc.vector.tensor_copy(
                out=agg_t[:].rearrange("p (t u) -> p t u", u=21)[:, :, 20:21],
                in_=valid_t[:][:, :, None])
            p6ctx = tc.tile_pool(name="p6", bufs=3, space="PSUM")
            p6pool = p6ctx.__enter__()
            # all gathers first, unimpeded: idx tiles DMA'd directly from DRAM,
            # each gather owns a persistent graw tile so queue-pairs generate
            # descriptors concurrently with downstream DVE/PE work.
            itks = []
            for k in range(KC):
                itk = cpool.tile([P, NI // 16], i16, tag=f"itk{k}",
                                 name=f"itk_{k}")
                nc.sync.dma_start(out=itk[:],
                                  in_=idx16[k * P:(k + 1) * P, :])
                itks.append(itk)
            for k in range(KC):
                graw = gpool.tile([P, 6 * 64], f32, tag="graw", bufs=8,
                                  name=f"graw_{k}")
                nc.gpsimd.dma_gather(
                    out_ap=graw[:].rearrange("p (n e) -> p n e", e=64),
                    in_ap=table[:],
                    idxs_ap=itks[k][:],
                    num_idxs=NI,
                    num_idxs_reg=NI,
                    elem_size=64,
                    queue_num=k % NSWQ,
                )
                msgs = gpool.tile([P, 6 * F], bf16, tag="msgs", bufs=4,
                                  name=f"msgs_{k}")
                nc.vector.tensor_tensor(
                    out=msgs[:].rearrange("p (t f) -> p t f", f=F),
                    in0=graw[:].rearrange("p (n e) -> p n e", e=64)[:, :, 0:F],
                    in1=norm_t[:, k * 6:(k + 1) * 6][:, :, None]
                        .to_broadcast([P, 6, F]),
                    op=OP.mult)
                oh = ohpool.tile([P, 6 * P], bf16, tag="oh", bufs=4,
                                 name=f"oh_{k}")
                nc.vector.tensor_tensor(
                    out=oh[:].rearrange("p (t q) -> p t q", q=P),
                    in0=dstrel_t[:, k * 6:(k + 1) * 6][:, :, None]
                        .to_broadcast([P, 6, P]),
                    in1=iota_t[:][:, None, :].to_broadcast([P, 6, P]),
                    op=OP.is_equal)
                ps6 = p6pool.tile([P, 120], f32, tag="ps6", name=f"ps6_{k}")
                for sl in range(6):
                    nc.tensor.matmul(
                        out=ps6[:, sl * F:(sl + 1) * F],
                        lhsT=oh[:, sl * P:(sl + 1) * P],
                        rhs=msgs[:, sl * F:(sl + 1) * F],
                        start=True, stop=True)
                nc.vector.tensor_tensor(
                    out=agg_t[:, k * 126:(k + 1) * 126]
                        .rearrange("p (s u) -> p s u", u=21)[:, :, 0:F],
                    in0=ps6[:].rearrange("p (s f) -> p s f", f=F),
                    in1=xl_t[:, k * 120:(k + 1) * 120]
                        .rearrange("p (s f) -> p s f", f=F),
                    op=OP.add)
                nc.vector.tensor_tensor(
                    out=aggp_t[:, k * 120:(k + 1) * 120],
                    in0=ps6[:],
                    in1=xl_t[:, k * 120:(k + 1) * 120],
                    op=OP.add)

            p6ctx.__exit__(None, None, None)

            # ---- Gram accumulation ----
            ggctx = tc.tile_pool(name="pgg", bufs=1, space="PSUM")
            ggpool = ggctx.__enter__()
            trctx = tc.tile_pool(name="ptr", bufs=2, space="PSUM")
            trpool = trctx.__enter__()
            gg_ps = ggpool.tile([126, 126], f32)
            for m in range(G6):
                nc.tensor.matmul(
                    out=gg_ps[:],
                    lhsT=agg_t[:, m * 126:(m + 1) * 126],
                    rhs=agg_t[:, m * 126:(m + 1) * 126],
                    start=(m == 0), stop=(m == G6 - 1))
            gg_sb = spool.tile([126, 126], f32)
            nc.vector.tensor_copy(out=gg_sb[:], in_=gg_ps[:])
            for i in range(6):
                nc.sync.dma_start(
                    out=ag_in[i],
                    in_=gg_sb[21 * i:21 * (i + 1), 21 * i:21 * i + 21])


            # ---- AllGather of Gram partials (issue early; overlaps PE) ----
            nc.gpsimd.collective_compute(
                "AllGather", OP.bypass,
                replica_groups=[list(range(C))],
                ins=[ag_in[:].opt()], outs=[ag_out[:].opt()])

            # ---- transposes for the final matmuls (independent of stats) ----
            trm_all = bpool.tile([120, G6 * P], f32)
            for m in range(G6):
                tr_ps = trpool.tile([120, P], f32, tag="trps", name=f"trps_{m}")
                nc.tensor.transpose(
                    out=tr_ps[:],
                    in_=aggp_t[:, m * 120:(m + 1) * 120],
                    identity=ident_t[:])
                nc.vector.tensor_copy(
                    out=trm_all[:, m * P:(m + 1) * P], in_=tr_ps[:])

            trctx.__exit__(None, None, None)
            ggctx.__exit__(None, None, None)

            # ---- fold AllGather result ----
            stctx = tc.tile_pool(name="pst", bufs=1, space="PSUM")
            stpool = stctx.__enter__()
            mpctx = tc.tile_pool(name="pmp", bufs=2, space="PSUM")
            mppool = mpctx.__enter__()
            lgctx = tc.tile_pool(name="plg", bufs=2, space="PSUM")
            lgpool = lgctx.__enter__()
            gsum_t = spool.tile([21, 48 * 21], f32)
            nc.sync.dma_start(
                out=gsum_t[:].rearrange("a (k b) -> a k b", b=21),
                in_=ag_out[:].rearrange("c s a b -> a (c s) b"))
            G1_t = spool.tile([21, 21], f32)
            nc.vector.reduce_sum(
                out=G1_t[:],
                in_=gsum_t[:].rearrange("a (k b) -> a b k", b=21),
                axis=AX)

            # ---- BN stats -> W_eff/b_eff ----
            w1aug_t = spool.tile([P, 8 * 21], f32)
            nc.vector.tensor_copy(
                out=w1aug_t[:].rearrange("p (c u) -> p c u", u=21)[:, :, 0:F],
                in_=WT8_t[:].rearrange("p (c f) -> p c f", f=F))
            nc.vector.tensor_copy(
                out=w1aug_t[:].rearrange("p (c u) -> p c u", u=21)[:, :, 20:21],
                in_=bcol8_t[:][:, :, None])
            wb_ps = stpool.tile([22, 2], f32, tag="wb", bufs=1)
            for c8 in range(8):
                m_ps = mppool.tile([P, 21], f32, tag="mps", name=f"mps_{c8}")
                nc.tensor.matmul(
                    out=m_ps[:], lhsT=W1_t[:, c8 * P:(c8 + 1) * P],
                    rhs=G1_t[:], start=True, stop=True)
                prod = spool.tile([P, 21], f32, tag="prod", name=f"prod_{c8}")
                nc.vector.tensor_tensor(
                    out=prod[:], in0=m_ps[:],
                    in1=w1aug_t[:, c8 * 21:(c8 + 1) * 21], op=OP.mult)
                ex2 = spool.tile([P, 1], f32, tag="ex2", name=f"ex2_{c8}")
                nc.vector.reduce_sum(out=ex2[:], in_=prod[:], axis=AX)
                mean = spool.tile([P, 1], f32, tag="mean", name=f"mean_{c8}")
                nc.vector.tensor_scalar_mul(
                    out=mean[:], in0=m_ps[:, 20:21], scalar1=1.0 / N)
                mm2 = spool.tile([P, 1], f32, tag="mm2", name=f"mm2_{c8}")
                nc.vector.tensor_tensor(
                    out=mm2[:], in0=mean[:], in1=mean[:], op=OP.mult)
                ex2n = spool.tile([P, 1], f32, tag="ex2n", name=f"ex2n_{c8}")
                nc.vector.tensor_scalar_mul(
                    out=ex2n[:], in0=ex2[:], scalar1=1.0 / N)
                var = spool.tile([P, 1], f32, tag="var", name=f"var_{c8}")
                nc.vector.tensor_tensor(
                    out=var[:], in0=ex2n[:], in1=mm2[:], op=OP.subtract)
                nc.vector.tensor_scalar_add(out=var[:], in0=var[:],
                                            scalar1=EPS)
                sd = spool.tile([P, 1], f32, tag="sd", name=f"sd_{c8}")
                nc.scalar.activation(out=sd[:], in_=var[:], func=ACT.Sqrt)
                dsc = spool.tile([P, 1], f32, tag="dsc", name=f"dsc_{c8}")
                nc.vector.reciprocal(out=dsc[:], in_=sd[:])
                nc.vector.tensor_tensor(
                    out=dsc[:], in0=dsc[:], in1=gamma8_t[:, c8:c8 + 1],
                    op=OP.mult)
                aug = spool.tile([P, 22], f32, tag="aug", name=f"aug_{c8}")
                nc.vector.tensor_scalar_mul(
                    out=aug[:, 0:F],
                    in0=WT8_t[:, c8 * F:(c8 + 1) * F], scalar1=dsc[:])
                bm = spool.tile([P, 1], f32, tag="bm", name=f"bm_{c8}")
                nc.vector.tensor_tensor(
                    out=bm[:], in0=bcol8_t[:, c8:c8 + 1], in1=mean[:],
                    op=OP.subtract)
                nc.vector.tensor_tensor(
                    out=aug[:, 20:21], in0=bm[:], in1=dsc[:], op=OP.mult)
                nc.vector.tensor_copy(
                    out=aug[:, 21:22], in_=beta8_t[:, c8:c8 + 1])
                nc.tensor.matmul(
                    out=wb_ps[:], lhsT=aug[:],
                    rhs=Wlin8_t[:, 2 * c8:2 * c8 + 2],
                    start=(c8 == 0), stop=(c8 == 7))
            rhs2 = spool.tile([22, 2], f32)
            nc.vector.tensor_tensor(
                out=rhs2[:], in0=wb_ps[:], in1=blin_t[:], op=OP.add)
            rhs_tiled = spool.tile([22, 12], f32)
            nc.vector.tensor_copy(
                out=rhs_tiled[:].rearrange("p (i o) -> p i o", o=2),
                in_=rhs2[:][:, None, :].to_broadcast([22, 6, 2]))
            wstack_ps = stpool.tile([120, 12], f32, tag="wstk", bufs=1)
            nc.tensor.matmul(out=wstack_ps[:], lhsT=SEL_t[:], rhs=rhs_tiled[:],
                             start=True, stop=True)
            wstack_t = spool.tile([120, 12], f32)
            nc.vector.tensor_tensor(out=wstack_t[:], in0=wstack_ps[:],
                                    in1=bmask_t[:], op=OP.mult)
            beff_ps = stpool.tile([P, 12], f32, tag="beff", bufs=1)
            nc.tensor.matmul(out=beff_ps[:], lhsT=SEL3_t[:], rhs=rhs_tiled[:],
                             start=True, stop=True)
            beff_t = spool.tile([P, 12], f32)
            nc.vector.tensor_copy(out=beff_t[:], in_=beff_ps[:])

            # ---- final matmuls + fused relu/softmax ----
            logits_t = bpool.tile([P, G6 * 12], f32)
            for m in range(G6):
                log_ps = lgpool.tile([P, 12], f32, tag="logps",
                                     name=f"logps_{m}")
                nc.tensor.matmul(out=log_ps[:],
                                 lhsT=trm_all[:, m * P:(m + 1) * P],
                                 rhs=wstack_t[:], start=True, stop=True)
                nc.vector.tensor_tensor(
                    out=logits_t[:, m * 12:(m + 1) * 12],
                    in0=log_ps[:], in1=beff_t[:], op=OP.add)
            esb = bpool.tile([P, G6 * 12], f32)
            nc.scalar.activation(out=esb[:], in_=logits_t[:], func=ACT.Exp)
            nc.vector.tensor_scalar_max(out=esb[:], in0=esb[:], scalar1=1.0)
            psum_t = spool.tile([P, T], f32)
            nc.vector.tensor_tensor(
                out=psum_t[:],
                in0=esb[:].rearrange("p (t o) -> p t o", o=2)[:, :, 0:1],
                in1=esb[:].rearrange("p (t o) -> p t o", o=2)[:, :, 1:2],
                op=OP.add)
            rc = spool.tile([P, T], f32)
            nc.vector.reciprocal(out=rc[:], in_=psum_t[:])
            outv = bpool.tile([P, T * 2], f32)
            nc.vector.tensor_tensor(
                out=outv[:].rearrange("p (t o) -> p t o", o=2),
                in0=esb[:].rearrange("p (t o) -> p t o", o=2),
                in1=rc[:][:, :, None].to_broadcast([P, T, 2]),
                op=OP.mult)
            nc.sync.dma_start(
                out=out_ext[:].rearrange("(t p) o -> p t o", p=P),
                in_=outv[:].rearrange("p (t o) -> p t o", o=2))
            if debug:
                nc.sync.dma_start(out=dbg_aggp[:], in_=aggp_t[:])
                nc.sync.dma_start(out=dbg_g1[:], in_=G1_t[:])
                nc.sync.dma_start(out=dbg_wstk[:], in_=wstack_t[:])
                nc.sync.dma_start(out=dbg_beff[:], in_=beff_t[:])
                nc.sync.dma_start(out=dbg_logit[:], in_=logits_t[:])
                nc.sync.dma_start(out=dbg_msgs[:], in_=aggp_t[:])
            lgctx.__exit__(None, None, None)
            mpctx.__exit__(None, None, None)
            stctx.__exit__(None, None, None)

    nc.finalize()
    return nc


# --------------------------------------------------------------------------
# entry point
# --------------------------------------------------------------------------
TRACE = False           # set True (e.g. from test.py) to neuron-profile the run
LAST_EXEC_NS = None


def kernel(**inputs):
    global LAST_EXEC_NS
    from concourse.bass_utils import run_bass_kernel_spmd

    per_core, shared, meta = _prep(**inputs)
    nc = _build(meta)
    in_maps = []
    for c in range(C):
        d = dict(per_core[c])
        m = {
            "table": d["table"], "idx16": d["idx16"], "normc": d["normc"],
            "dstrel": d["dstrel"],
            "x_local": np.ascontiguousarray(
                d["x_local"].reshape(P, meta["T"] * F)),
            "dinv2": d["dinv2"], "valid": d["valid"],
            "W1": shared["W1"],
            "WT8": np.ascontiguousarray(shared["WT8"].reshape(P, 8 * F)),
            "bcol8": shared["bcol8"], "beta8": shared["beta8"],
            "gamma8": shared["gamma8"],
            "W_lin8": np.ascontiguousarray(shared["W_lin8"].reshape(P, 16)),
            "blin_pad": shared["blin_pad"], "SEL": shared["SEL"],
            "SEL3": shared["SEL3"], "BMASK": shared["BMASK"],
            "iota_bc": shared["iota_bc"],
            "identity": shared["identity"],
        }
        in_maps.append(m)
    res = run_bass_kernel_spmd(nc, in_maps, core_ids=list(range(C)),
                               trace=TRACE)
    LAST_EXEC_NS = res.exec_time_ns
    outs = [res.results[c]["out"] for c in range(C)]
    stacked = np.stack(outs)
    full = stacked[meta["core_of_node"], meta["slot_of_node"]]
    return np.ascontiguousarray(full.astype(np.float32))


# revision 18
# speedup vs baseline: 2.7083x; 1.8011x over previous
"""Distributed Trainium2 Bass kernel for nn_ActorGCN (GCN message passing).

Strategy (8 NeuronCores, nodes sharded across cores):
  The reference computes softmax(relu(BN(GCNConv(x)) @ W_lin)).  Because the
  GCN aggregation is linear, we aggregate FIRST on the 20-dim raw features
  (agg = A_norm @ x), then fold the whole 1024-wide hidden layer analytically:
  BatchNorm statistics of h = agg @ W + b are exact functions of the 21x21
  Gram matrix [agg,1]^T [agg,1], so the final output is
  softmax(relu(agg @ W_eff + b_eff)) with a tiny on-device-computed
  W_eff [20,2].  Per-core work: one dma_gather of source-node features
  (compacted per-core table, int16 indices), scale by the symmetric-norm
  edge coefficients, one-hot segmented-sum matmuls into PSUM (one 128-slot
  group per 128-row dst tile, guaranteed by degree-balanced node dealing),
  Gram accumulation, one small AllGather, stats fold, and the final
  [128,20]x[20,2] matmuls with fused relu/softmax.

Host-side prep is index-space only: degrees, norm coefficients, the
node->(core,tile,row) assignment, slot layouts and gather tables.
"""
import numpy as np
import ml_dtypes

F = 20
C = 8
P = 128
EPS = 1e-5
NQ = 4          # gather chunks
NSWQ = 4        # swdge queues (call k uses queue k % NSWQ)


# --------------------------------------------------------------------------
# host-side preprocessing (index space only)
# --------------------------------------------------------------------------
def _prep(state, edge_attr, edge_index, W_gcn, b_gcn, gamma, beta, W_lin, b_lin):
    N = state.shape[0] + edge_attr.shape[0]
    x_full = np.concatenate([np.asarray(state, np.float32),
                             np.asarray(edge_attr, np.float32)], axis=0)
    src = np.asarray(edge_index[0]).astype(np.int64)
    dst = np.asarray(edge_index[1]).astype(np.int64)

    deg_in = np.bincount(dst, minlength=N)
    deg = (deg_in + 1).astype(np.float32)
    dinv = (1.0 / np.sqrt(deg)).astype(np.float32)
    norm = (dinv[src] * dinv[dst]).astype(np.float32)
    dinv2 = (dinv * dinv).astype(np.float32)

    # degree-balanced node dealing over C*T bins of P rows each
    T = -(-N // (C * P))
    NB = C * T
    SHP = T * P
    order = np.argsort(-deg_in, kind="stable")
    nrounds = -(-N // NB)
    bin_of_node = np.empty(N, dtype=np.int64)
    for r in range(nrounds):
        lo, hi = r * NB, min((r + 1) * NB, N)
        seq = np.arange(hi - lo)
        b = seq if (r % 2 == 0) else (NB - 1 - seq)
        bin_of_node[order[lo:hi]] = b
    load = np.bincount(bin_of_node, weights=deg_in.astype(np.float64),
                       minlength=NB).astype(np.int64)
    if load.max() > P:
        zero_nodes = [list() for _ in range(NB)]
        for n in np.nonzero(deg_in == 0)[0]:
            zero_nodes[bin_of_node[n]].append(int(n))
        for b in np.nonzero(load > P)[0]:
            members = np.nonzero(bin_of_node == b)[0]
            members = list(members[np.argsort(deg_in[members])])
            while load[b] > P:
                pos = next(int(n) for n in members if deg_in[n] > 0)
                members.remove(pos)
                tgt = int(np.argmin(load + np.array(
                    [0 if zero_nodes[i] else 10**9 for i in range(NB)])))
                z = zero_nodes[tgt].pop()
                bin_of_node[pos], bin_of_node[z] = tgt, b
                zero_nodes[b].append(z)
                load[b] -= deg_in[pos]
                load[tgt] += deg_in[pos]
        assert load.max() <= P, load.max()

    ord2 = np.lexsort((np.arange(N), bin_of_node))
    row_in_bin = np.empty(N, dtype=np.int64)
    cnt_per_bin = np.bincount(bin_of_node, minlength=NB)
    assert cnt_per_bin.max() <= P
    starts = np.zeros(NB + 1, dtype=np.int64)
    np.cumsum(cnt_per_bin, out=starts[1:])
    row_in_bin[ord2] = np.arange(N) - starts[bin_of_node[ord2]]

    core_of_node = bin_of_node // T
    tile_of_node = bin_of_node % T
    slot_of_node = tile_of_node * P + row_in_bin

    node_at = np.full((C, SHP), -1, dtype=np.int64)
    node_at[core_of_node, slot_of_node] = np.arange(N)

    S = SHP
    ec = core_of_node[dst]
    et = tile_of_node[dst]
    erel = slot_of_node[dst] % P

    core_data = []
    U_list = []
    for c in range(C):
        m = ec == c
        s_c, t_c, rel_c, n_c = src[m], et[m], erel[m], norm[m]
        o = np.lexsort((s_c, t_c))
        s_c, t_c, rel_c, n_c = s_c[o], t_c[o], rel_c[o], n_c[o]
        uniq, idx_local = np.unique(s_c, return_inverse=True)
        U_list.append(len(uniq))
        core_data.append((s_c, t_c, rel_c, n_c, uniq, idx_local))
    U_pad = max(U_list)
    U_pad = -(-U_pad // 4) * 4
    assert U_pad < 32767, U_pad

    per_core = []
    for c in range(C):
        s_c, t_c, rel_c, n_c, uniq, idx_local = core_data[c]
        cnt = np.bincount(t_c, minlength=T)
        assert cnt.max() <= P
        cbase = np.zeros(T + 1, dtype=np.int64)
        np.cumsum(cnt, out=cbase[1:])
        slot = P * t_c + (np.arange(len(t_c)) - cbase[t_c])

        msg_idx = np.zeros(S, dtype=np.int16)
        msg_norm = np.zeros(S, dtype=np.float32)
        msg_dstrel = np.zeros(S, dtype=np.float32)
        msg_idx[slot] = idx_local.astype(np.int16)
        msg_norm[slot] = n_c
        msg_dstrel[slot] = rel_c.astype(np.float32)

        table = np.zeros((U_pad, 64), dtype=np.float32)
        table[:len(uniq), :F] = x_full[uniq]

        def slotted(a):
            return np.ascontiguousarray(a.reshape(S // P, P).T)

        # per-call idx wraps: call k covers slots [k*NI, (k+1)*NI)
        NI = 768
        KC = S // NI
        iws = []
        for k in range(KC):
            blk = msg_idx[k * NI:(k + 1) * NI]
            iws.append(np.tile(blk.reshape(NI // 16, 16).T, (8, 1)))
        iwrap = np.ascontiguousarray(np.stack(iws).reshape(KC * 128, NI // 16))

        nodes = node_at[c]
        ok = nodes >= 0
        xl = np.zeros((SHP, F), dtype=np.float32)
        xl[ok] = x_full[nodes[ok]]
        xl = np.ascontiguousarray(xl.reshape(T, P, F).transpose(1, 0, 2))
        d2 = np.zeros(SHP, dtype=np.float32)
        d2[ok] = dinv2[nodes[ok]]
        d2 = np.ascontiguousarray(d2.reshape(T, P).T)
        vv = np.ascontiguousarray(ok.astype(np.float32).reshape(T, P).T)

        per_core.append(dict(
            table=table, idx16=iwrap, normc=slotted(msg_norm),
            dstrel=slotted(msg_dstrel).astype(ml_dtypes.bfloat16),
            x_local=xl, dinv2=d2, valid=vv,
        ))

    H = W_gcn.shape[1]
    W1 = np.concatenate([np.asarray(W_gcn, np.float32),
                         np.asarray(b_gcn, np.float32)[None, :]], axis=0)
    WT8 = np.ascontiguousarray(np.asarray(W_gcn, np.float32).T
                               .reshape(8, P, F).transpose(1, 0, 2))
    def col8(v):
        return np.ascontiguousarray(np.asarray(v, np.float32).reshape(8, P).T)
    W_lin8 = np.ascontiguousarray(np.asarray(W_lin, np.float32)
                                  .reshape(8, P, 2).transpose(1, 0, 2))
    blin_pad = np.zeros((22, 2), dtype=np.float32)
    blin_pad[21, :] = np.asarray(b_lin, np.float32)

    SEL = np.zeros((22, 120), dtype=np.float32)
    for i in range(6):
        for a in range(20):
            SEL[a, 20 * i + a] = 1.0
    BMASK = np.zeros((120, 12), dtype=np.float32)
    for i in range(6):
        BMASK[20 * i:20 * (i + 1), 2 * i:2 * (i + 1)] = 1.0
    SEL3 = np.zeros((22, 128), dtype=np.float32)
    SEL3[20, :] = 1.0
    SEL3[21, :] = 1.0
    iota_bc = np.tile(np.arange(P, dtype=np.float32)[None, :], (P, 1)) \
        .astype(ml_dtypes.bfloat16)
    identity = np.eye(P, dtype=np.float32)

    shared = dict(W1=W1, WT8=WT8, bcol8=col8(b_gcn), beta8=col8(beta),
                  gamma8=col8(gamma), W_lin8=W_lin8, blin_pad=blin_pad,
                  SEL=SEL, SEL3=SEL3, BMASK=BMASK, iota_bc=iota_bc,
                  identity=identity)
    meta = dict(N=N, T=T, SHP=SHP, S=S, U_pad=U_pad, H=H,
                core_of_node=core_of_node, slot_of_node=slot_of_node)
    return per_core, shared, meta


# --------------------------------------------------------------------------
# device kernel
# --------------------------------------------------------------------------
def _build(meta, debug=False):
    import concourse.bass as bass
    import concourse.bacc as bacc
    import concourse.mybir as mybir
    from concourse.tile import TileContext

    f32 = mybir.dt.float32
    bf16 = mybir.dt.bfloat16
    i16 = mybir.dt.int16
    T, S, U_pad, N = meta["T"], meta["S"], meta["U_pad"], meta["N"]
    G6 = T // 6                      # 6-tile groups (34)
    AX = mybir.AxisListType.X
    OP = mybir.AluOpType
    ACT = mybir.ActivationFunctionType

    nc = bacc.Bacc(None, target_bir_lowering=False,
                   num_swdge_queues=NSWQ)

    def inp(name, shape, dt=f32):
        return nc.declare_dram_parameter(name, list(shape), dt, isOutput=False)

    table = inp("table", [U_pad, 64])
    NI = 768                     # idxs per dma_gather call
    KC = S // NI                 # calls (34), one per 6-tile group
    idx16 = inp("idx16", [KC * P, NI // 16], i16)
    normc = inp("normc", [P, T])
    dstrel = inp("dstrel", [P, T], bf16)
    x_local = inp("x_local", [P, T * F])
    dinv2 = inp("dinv2", [P, T])
    valid = inp("valid", [P, T])
    W1 = inp("W1", [21, 1024])
    WT8 = inp("WT8", [P, 8 * F])
    bcol8 = inp("bcol8", [P, 8])
    beta8 = inp("beta8", [P, 8])
    gamma8 = inp("gamma8", [P, 8])
    W_lin8 = inp("W_lin8", [P, 16])
    blin_pad = inp("blin_pad", [22, 2])
    SEL = inp("SEL", [22, 120])
    SEL3 = inp("SEL3", [22, 128])
    BMASK = inp("BMASK", [120, 12])
    iota_bc = inp("iota_bc", [P, P], bf16)
    identity = inp("identity", [P, P])
    out_ext = nc.declare_dram_parameter("out", [P, (S // P) * 2], f32,
                                        isOutput=True)
    if debug:
        dbg_aggp = nc.declare_dram_parameter("dbg_aggp", [P, T * F], f32,
                                             isOutput=True)
        dbg_g1 = nc.declare_dram_parameter("dbg_g1", [21, 21], f32,
                                           isOutput=True)
        dbg_wstk = nc.declare_dram_parameter("dbg_wstk", [120, 12], f32,
                                             isOutput=True)
        dbg_beff = nc.declare_dram_parameter("dbg_beff", [P, 12], f32,
                                             isOutput=True)
        dbg_logit = nc.declare_dram_parameter("dbg_logit", [P, T * 2], f32,
                                              isOutput=True)
        dbg_msgs = nc.declare_dram_parameter("dbg_msgs", [P, T * F], f32,
                                             isOutput=True)

    with TileContext(nc) as tc:
        with (
            tc.tile_pool(name="dram", bufs=1, space="DRAM") as dpool,
            tc.tile_pool(name="const", bufs=1) as cpool,
            tc.tile_pool(name="big", bufs=1) as bpool,
            tc.tile_pool(name="graw", bufs=2) as gpool,
            tc.tile_pool(name="oh", bufs=2) as ohpool,
            tc.tile_pool(name="small", bufs=2) as spool,
        ):
            ag_in = dpool.tile([6, 21, 21], f32, tag="ag_in", name="ag_in")
            ag_out = dpool.tile([8, 6, 21, 21], f32, tag="ag_out",
                                name="ag_out", addr_space="Shared")

            # ---- load inputs ----
            def load(nm, ap, shape, dt=f32, pool=cpool):
                t = pool.tile(list(shape), dt, tag=nm, name=nm)
                nc.sync.dma_start(out=t[:], in_=ap[:])
                return t


            # idx tiles + gather-critical inputs first so the gathers start
            # within a few us; everything else loads behind them.
            itks = []
            for k in range(KC):
                itk = cpool.tile([P, NI // 16], i16, tag=f"itk{k}",
                                 name=f"itk_{k}")
                nc.sync.dma_start(out=itk[:],
                                  in_=idx16[k * P:(k + 1) * P, :])
                itks.append(itk)
            norm_t = load("norm_t", normc, [P, T])
            dstrel_t = load("dstrel_t", dstrel, [P, T], bf16)
            iota_t = load("iota_t", iota_bc, [P, P], bf16)
            graws = []
            for k in range(KC):
                graw = gpool.tile([P, 6 * 64], f32, tag="graw", bufs=8,
                                  name=f"graw_{k}")
                nc.gpsimd.dma_gather(
                    out_ap=graw[:].rearrange("p (n e) -> p n e", e=64),
                    in_ap=table[:],
                    idxs_ap=itks[k][:],
                    num_idxs=NI,
                    num_idxs_reg=NI,
                    elem_size=64,
                    queue_num=k % NSWQ,
                )
                graws.append(graw)
            ohs = []
            for k in range(KC):
                oh = ohpool.tile([P, 6 * P], bf16, tag="oh", bufs=8,
                                 name=f"oh_{k}")
                nc.vector.tensor_tensor(
                    out=oh[:].rearrange("p (t q) -> p t q", q=P),
                    in0=dstrel_t[:, k * 6:(k + 1) * 6][:, :, None]
                        .to_broadcast([P, 6, P]),
                    in1=iota_t[:][:, None, :].to_broadcast([P, 6, P]),
                    op=OP.is_equal)
                ohs.append(oh)
            xl_t = load("xl_t", x_local, [P, T * F], pool=bpool)
            d2_t = load("d2_t", dinv2, [P, T])
            valid_t = load("valid_t", valid, [P, T])
            W1_t = load("W1_t", W1, [21, 1024])
            WT8_t = load("WT8_t", WT8, [P, 8 * F])
            bcol8_t = load("bcol8_t", bcol8, [P, 8])
            beta8_t = load("beta8_t", beta8, [P, 8])
            gamma8_t = load("gamma8_t", gamma8, [P, 8])
            Wlin8_t = load("Wlin8_t", W_lin8, [P, 16])
            blin_t = load("blin_t", blin_pad, [22, 2])
            SEL_t = load("SEL_t", SEL, [22, 120])
            SEL3_t = load("SEL3_t", SEL3, [22, 128])
            bmask_t = load("bmask_t", BMASK, [120, 12])
            ident_t = load("ident_t", identity, [P, P])

            # ---- self-loop term (in place into xl_t) ----
            nc.vector.tensor_tensor(
                out=xl_t[:].rearrange("p (t f) -> p t f", f=F),
                in0=xl_t[:].rearrange("p (t f) -> p t f", f=F),
                in1=d2_t[:][:, :, None].to_broadcast([P, T, F]),
                op=OP.mult)

            # ---- agg via gather + one-hot segmented-sum matmuls ----
            # One dma_gather call per 6-tile group (NI=768 idxs; the SWDGE
            # descriptor ring caps ~1k descriptors per instruction).
            # agg_t: [tile|valid]-interleaved (21 cols/tile) for the Gram;
            # aggp_t: packed 20 cols/tile for the PE transposes.
            agg_t = bpool.tile([P, T * 21], f32)
            aggp_t = bpool.tile([P, T * F], f32)
            nc.vector.tensor_copy(
                out=agg_t[:].rearrange("p (t u) -> p t u", u=21)[:, :, 20:21],
                in_=valid_t[:][:, :, None])
            p6ctx = tc.tile_pool(name="p6", bufs=3, space="PSUM")
            p6pool = p6ctx.__enter__()
            # all gathers first, unimpeded: idx tiles DMA'd directly from DRAM,
            # each gather owns a persistent graw tile so queue-pairs generate
            # descriptors concurrently with downstream DVE/PE work.
            for k in range(KC):
                graw = graws[k]
                msgs = gpool.tile([P, 6 * F], bf16, tag="msgs", bufs=8,
                                  name=f"msgs_{k}")
                nc.vector.tensor_tensor(
                    out=msgs[:].rearrange("p (t f) -> p t f", f=F),
                    in0=graw[:].rearrange("p (n e) -> p n e", e=64)[:, :, 0:F],
                    in1=norm_t[:, k * 6:(k + 1) * 6][:, :, None]
                        .to_broadcast([P, 6, F]),
                    op=OP.mult)
                oh = ohs[k]
                ps6 = p6pool.tile([P, 120], f32, tag="ps6", name=f"ps6_{k}")
                for sl in range(6):
                    nc.tensor.matmul(
                        out=ps6[:, sl * F:(sl + 1) * F],
                        lhsT=oh[:, sl * P:(sl + 1) * P],
                        rhs=msgs[:, sl * F:(sl + 1) * F],
                        start=True, stop=True)
                nc.vector.tensor_tensor(
                    out=agg_t[:, k * 126:(k + 1) * 126]
                        .rearrange("p (s u) -> p s u", u=21)[:, :, 0:F],
                    in0=ps6[:].rearrange("p (s f) -> p s f", f=F),
                    in1=xl_t[:, k * 120:(k + 1) * 120]
                        .rearrange("p (s f) -> p s f", f=F),
                    op=OP.add)
                nc.vector.tensor_tensor(
                    out=aggp_t[:, k * 120:(k + 1) * 120],
                    in0=ps6[:],
                    in1=xl_t[:, k * 120:(k + 1) * 120],
                    op=OP.add)

            p6ctx.__exit__(None, None, None)

            # ---- Gram accumulation ----
            ggctx = tc.tile_pool(name="pgg", bufs=1, space="PSUM")
            ggpool = ggctx.__enter__()
            trctx = tc.tile_pool(name="ptr", bufs=2, space="PSUM")
            trpool = trctx.__enter__()
            gg_ps = ggpool.tile([126, 126], f32)
            for m in range(G6):
                nc.tensor.matmul(
                    out=gg_ps[:],
                    lhsT=agg_t[:, m * 126:(m + 1) * 126],
                    rhs=agg_t[:, m * 126:(m + 1) * 126],
                    start=(m == 0), stop=(m == G6 - 1))
            gg_sb = spool.tile([126, 126], f32)
            nc.vector.tensor_copy(out=gg_sb[:], in_=gg_ps[:])
            for i in range(6):
                nc.sync.dma_start(
                    out=ag_in[i],
                    in_=gg_sb[21 * i:21 * (i + 1), 21 * i:21 * i + 21])


            # ---- AllGather of Gram partials (issue early; overlaps PE) ----
            nc.gpsimd.collective_compute(
                "AllGather", OP.bypass,
                replica_groups=[list(range(C))],
                ins=[ag_in[:].opt()], outs=[ag_out[:].opt()])

            # ---- transposes for the final matmuls (independent of stats) ----
            trm_all = bpool.tile([120, G6 * P], f32)
            for m in range(G6):
                tr_ps = trpool.tile([120, P], f32, tag="trps", name=f"trps_{m}")
                nc.tensor.transpose(
                    out=tr_ps[:],
                    in_=aggp_t[:, m * 120:(m + 1) * 120],
                    identity=ident_t[:])
                nc.vector.tensor_copy(
                    out=trm_all[:, m * P:(m + 1) * P], in_=tr_ps[:])

            trctx.__exit__(None, None, None)
            ggctx.__exit__(None, None, None)

            # ---- fold AllGather result ----
            stctx = tc.tile_pool(name="pst", bufs=1, space="PSUM")
            stpool = stctx.__enter__()
            mpctx = tc.tile_pool(name="pmp", bufs=2, space="PSUM")
            mppool = mpctx.__enter__()
            lgctx = tc.tile_pool(name="plg", bufs=2, space="PSUM")
            lgpool = lgctx.__enter__()
            gsum_t = spool.tile([21, 48 * 21], f32)
            nc.sync.dma_start(
                out=gsum_t[:].rearrange("a (k b) -> a k b", b=21),
                in_=ag_out[:].rearrange("c s a b -> a (c s) b"))
            G1_t = spool.tile([21, 21], f32)
            nc.vector.reduce_sum(
                out=G1_t[:],
                in_=gsum_t[:].rearrange("a (k b) -> a b k", b=21),
                axis=AX)

            # ---- BN stats -> W_eff/b_eff ----
            w1aug_t = spool.tile([P, 8 * 21], f32)
            nc.vector.tensor_copy(
                out=w1aug_t[:].rearrange("p (c u) -> p c u", u=21)[:, :, 0:F],
                in_=WT8_t[:].rearrange("p (c f) -> p c f", f=F))
            nc.vector.tensor_copy(
                out=w1aug_t[:].rearrange("p (c u) -> p c u", u=21)[:, :, 20:21],
                in_=bcol8_t[:][:, :, None])
            wb_ps = stpool.tile([22, 2], f32, tag="wb", bufs=1)
            for c8 in range(8):
                m_ps = mppool.tile([P, 21], f32, tag="mps", name=f"mps_{c8}")
                nc.tensor.matmul(
                    out=m_ps[:], lhsT=W1_t[:, c8 * P:(c8 + 1) * P],
                    rhs=G1_t[:], start=True, stop=True)
                prod = spool.tile([P, 21], f32, tag="prod", name=f"prod_{c8}")
                nc.vector.tensor_tensor(
                    out=prod[:], in0=m_ps[:],
                    in1=w1aug_t[:, c8 * 21:(c8 + 1) * 21], op=OP.mult)
                ex2 = spool.tile([P, 1], f32, tag="ex2", name=f"ex2_{c8}")
                nc.vector.reduce_sum(out=ex2[:], in_=prod[:], axis=AX)
                mean = spool.tile([P, 1], f32, tag="mean", name=f"mean_{c8}")
                nc.vector.tensor_scalar_mul(
                    out=mean[:], in0=m_ps[:, 20:21], scalar1=1.0 / N)
                mm2 = spool.tile([P, 1], f32, tag="mm2", name=f"mm2_{c8}")
                nc.vector.tensor_tensor(
                    out=mm2[:], in0=mean[:], in1=mean[:], op=OP.mult)
                ex2n = spool.tile([P, 1], f32, tag="ex2n", name=f"ex2n_{c8}")
                nc.vector.tensor_scalar_mul(
                    out=ex2n[:], in0=ex2[:], scalar1=1.0 / N)
                var = spool.tile([P, 1], f32, tag="var", name=f"var_{c8}")
                nc.vector.tensor_tensor(
                    out=var[:], in0=ex2n[:], in1=mm2[:], op=OP.subtract)
                nc.vector.tensor_scalar_add(out=var[:], in0=var[:],
                                            scalar1=EPS)
                sd = spool.tile([P, 1], f32, tag="sd", name=f"sd_{c8}")
                nc.scalar.activation(out=sd[:], in_=var[:], func=ACT.Sqrt)
                dsc = spool.tile([P, 1], f32, tag="dsc", name=f"dsc_{c8}")
                nc.vector.reciprocal(out=dsc[:], in_=sd[:])
                nc.vector.tensor_tensor(
                    out=dsc[:], in0=dsc[:], in1=gamma8_t[:, c8:c8 + 1],
                    op=OP.mult)
                aug = spool.tile([P, 22], f32, tag="aug", name=f"aug_{c8}")
                nc.vector.tensor_scalar_mul(
                    out=aug[:, 0:F],
                    in0=WT8_t[:, c8 * F:(c8 + 1) * F], scalar1=dsc[:])
                bm = spool.tile([P, 1], f32, tag="bm", name=f"bm_{c8}")
                nc.vector.tensor_tensor(
                    out=bm[:], in0=bcol8_t[:, c8:c8 + 1], in1=mean[:],
                    op=OP.subtract)
                nc.vector.tensor_tensor(
                    out=aug[:, 20:21], in0=bm[:], in1=dsc[:], op=OP.mult)
                nc.vector.tensor_copy(
                    out=aug[:, 21:22], in_=beta8_t[:, c8:c8 + 1])
                nc.tensor.matmul(
                    out=wb_ps[:], lhsT=aug[:],
                    rhs=Wlin8_t[:, 2 * c8:2 * c8 + 2],
                    start=(c8 == 0), stop=(c8 == 7))
            rhs2 = spool.tile([22, 2], f32)
            nc.vector.tensor_tensor(
                out=rhs2[:], in0=wb_ps[:], in1=blin_t[:], op=OP.add)
            rhs_tiled = spool.tile([22, 12], f32)
            nc.vector.tensor_copy(
                out=rhs_tiled[:].rearrange("p (i o) -> p i o", o=2),
                in_=rhs2[:][:, None, :].to_broadcast([22, 6, 2]))
            wstack_ps = stpool.tile([120, 12], f32, tag="wstk", bufs=1)
            nc.tensor.matmul(out=wstack_ps[:], lhsT=SEL_t[:], rhs=rhs_tiled[:],
                             start=True, stop=True)
            wstack_t = spool.tile([120, 12], f32)
            nc.vector.tensor_tensor(out=wstack_t[:], in0=wstack_ps[:],
                                    in1=bmask_t[:], op=OP.mult)
            beff_ps = stpool.tile([P, 12], f32, tag="beff", bufs=1)
            nc.tensor.matmul(out=beff_ps[:], lhsT=SEL3_t[:], rhs=rhs_tiled[:],
                             start=True, stop=True)
            beff_t = spool.tile([P, 12], f32)
            nc.vector.tensor_copy(out=beff_t[:], in_=beff_ps[:])

            # ---- final matmuls + fused relu/softmax ----
            logits_t = bpool.tile([P, G6 * 12], f32)
            for m in range(G6):
                log_ps = lgpool.tile([P, 12], f32, tag="logps",
                                     name=f"logps_{m}")
                nc.tensor.matmul(out=log_ps[:],
                                 lhsT=trm_all[:, m * P:(m + 1) * P],
                                 rhs=wstack_t[:], start=True, stop=True)
                nc.vector.tensor_tensor(
                    out=logits_t[:, m * 12:(m + 1) * 12],
                    in0=log_ps[:], in1=beff_t[:], op=OP.add)
            esb = bpool.tile([P, G6 * 12], f32)
            nc.scalar.activation(out=esb[:], in_=logits_t[:], func=ACT.Exp)
            nc.vector.tensor_scalar_max(out=esb[:], in0=esb[:], scalar1=1.0)
            psum_t = spool.tile([P, T], f32)
            nc.vector.tensor_tensor(
                out=psum_t[:],
                in0=esb[:].rearrange("p (t o) -> p t o", o=2)[:, :, 0:1],
                in1=esb[:].rearrange("p (t o) -> p t o", o=2)[:, :, 1:2],
                op=OP.add)
            rc = spool.tile([P, T], f32)
            nc.vector.reciprocal(out=rc[:], in_=psum_t[:])
            outv = bpool.tile([P, T * 2], f32)
            nc.vector.tensor_tensor(
                out=outv[:].rearrange("p (t o) -> p t o", o=2),
                in0=esb[:].rearrange("p (t o) -> p t o", o=2),
                in1=rc[:][:, :, None].to_broadcast([P, T, 2]),
                op=OP.mult)
            nc.sync.dma_start(out=out_ext[:], in_=outv[:])
            if debug:
                nc.sync.dma_start(out=dbg_aggp[:], in_=aggp_t[:])
                nc.sync.dma_start(out=dbg_g1[:], in_=G1_t[:])
                nc.sync.dma_start(out=dbg_wstk[:], in_=wstack_t[:])
                nc.sync.dma_start(out=dbg_beff[:], in_=beff_t[:])
                nc.sync.dma_start(out=dbg_logit[:], in_=logits_t[:])
                nc.sync.dma_start(out=dbg_msgs[:], in_=aggp_t[:])
            lgctx.__exit__(None, None, None)
            mpctx.__exit__(None, None, None)
            stctx.__exit__(None, None, None)

    nc.finalize()
    return nc


# --------------------------------------------------------------------------
# entry point
# --------------------------------------------------------------------------
TRACE = False           # set True (e.g. from test.py) to neuron-profile the run
LAST_EXEC_NS = None


def kernel(**inputs):
    global LAST_EXEC_NS
    from concourse.bass_utils import run_bass_kernel_spmd

    per_core, shared, meta = _prep(**inputs)
    nc = _build(meta)
    in_maps = []
    for c in range(C):
        d = dict(per_core[c])
        m = {
            "table": d["table"], "idx16": d["idx16"], "normc": d["normc"],
            "dstrel": d["dstrel"],
            "x_local": np.ascontiguousarray(
                d["x_local"].reshape(P, meta["T"] * F)),
            "dinv2": d["dinv2"], "valid": d["valid"],
            "W1": shared["W1"],
            "WT8": np.ascontiguousarray(shared["WT8"].reshape(P, 8 * F)),
            "bcol8": shared["bcol8"], "beta8": shared["beta8"],
            "gamma8": shared["gamma8"],
            "W_lin8": np.ascontiguousarray(shared["W_lin8"].reshape(P, 16)),
            "blin_pad": shared["blin_pad"], "SEL": shared["SEL"],
            "SEL3": shared["SEL3"], "BMASK": shared["BMASK"],
            "iota_bc": shared["iota_bc"],
            "identity": shared["identity"],
        }
        in_maps.append(m)
    res = run_bass_kernel_spmd(nc, in_maps, core_ids=list(range(C)),
                               trace=TRACE)
    LAST_EXEC_NS = res.exec_time_ns
    T = meta["T"]
    outs = [res.results[c]["out"].reshape(P, T, 2).transpose(1, 0, 2)
            .reshape(T * P, 2) for c in range(C)]
    stacked = np.stack(outs)
    full = stacked[meta["core_of_node"], meta["slot_of_node"]]
    return np.ascontiguousarray(full.astype(np.float32))
